# revision 15
# baseline (speedup 1.0000x reference)
"""Trainium2 Bass kernel for the EnhancedMathematicalReasoning MoE-routing module.

Computation (per token t, hidden dim H=2048, E=8 experts, dense routing):
    a1     = gelu(h @ Wd1 + bd1)
    logits = a1 @ Wd2 + bd2
    op_w   = softmax(logits)
    comb   = sum_e op_w[:, e] * (h @ We[e] + be[e])
    out    = (gelu(comb @ Wi1 + bi1) @ Wi2 + bi2) * mask

Sharding: data-parallel over the 8192 tokens -> 1024 tokens per NeuronCore,
weights replicated, no collectives.

Per-core strategy (P=128), v2 -- all GEMM operands in bf16:
  - Host casts h + all weights to bf16.  PE streaming rate is identical to
    f32r (1 cycle/row), but bf16 enables FWL fast weight loads (the f32r
    LDWEIGHTS was the exposed +14.5ns/MM tax in v1), halves all DMA traffic
    and makes PE transposes 1.0 cyc/row instead of 1.5.
  - h is PE-transposed once to hT [H, T] (bf16).  GEMM1/experts/GEMM3 run
    with the weight m-chunk stationary and a resident [H,*] activation as a
    512-wide moving operand, accumulating over K=16 chunks in PSUM.
  - Logits are accumulated in [E, T] orientation in a single PSUM bank per
    T-half across all 16 m-chunks (stationary = Wd2 m-slice [128, 8]), so
    softmax needs no transposes: exp on ACT (bias=bd2, no max subtraction --
    logits have sigma ~0.6), partition-sum + reciprocal-broadcast via tiny
    K=8/K=1 matmuls, one DVE multiply -> normalized op_w in [E, T].
  - Expert combine: op_w row e is broadcast to 128 partitions by a K=8
    selector matmul; DVE does comb += psum * ob into a bf16 arena [H, T].
  - GEMM1 loops c-outer / T-half-inner so each Wd1 column chunk covers
    ~28us of PE work (DMA deadlines trivially met) and the kernel starts as
    soon as the first 512 tokens are transposed; the remaining h transposes
    interleave into chunk 0's first column of psums.
  - GEMM4 is flipped: stationary = a2T token-slice [128k, 128t], moving =
    Wi2 column-chunk -> PSUM is directly [T, H]-oriented, the attention
    mask is fused into the eviction as a per-partition ACT scale, and the
    result DMAs straight out.  No output transposes, ~1.5us tail.
"""

import numpy as np
from contextlib import ExitStack

import ml_dtypes

import concourse.bass as bass
import concourse.tile as tile
from concourse import bacc, mybir
from concourse.bass_utils import run_bass_kernel_spmd
from concourse.masks import make_identity

F32 = mybir.dt.float32
F32R = mybir.dt.float32r
BF16 = mybir.dt.bfloat16
AF = mybir.ActivationFunctionType
ALU = mybir.AluOpType
AX = mybir.AxisListType

P = 128
N_CORES = 8

B, S, H_FULL, E_FULL = 4, 2048, 2048, 8


def build_nc(T, H, E, act=AF.Gelu, include_be=False, include_bi2=False):
    """Build + compile the single-core program (same program runs SPMD on all
    cores). T: tokens per core. Requires T % 1024 == 0, H % 512 == 0."""
    assert T % 1024 == 0 and H % 512 == 0 and E <= P
    KT = H // P      # k-chunks of the contraction dim
    TT = T // P      # token 128-blocks
    TB = T // 512    # token 512-blocks
    MT = H // P      # output m-chunks
    C = H // 512     # 512-wide weight column chunks

    nc = bacc.Bacc("TRN2", target_bir_lowering=False, debug=False)

    h_d = nc.dram_tensor("h", [T, H], BF16, kind="ExternalInput").ap()
    msk_d = nc.dram_tensor("mask", [T], F32, kind="ExternalInput").ap()
    wd1_d = nc.dram_tensor("wd1", [H, H], F32R, kind="ExternalInput").ap()
    bd1_d = nc.dram_tensor("bd1", [H], F32, kind="ExternalInput").ap()
    wd2_d = nc.dram_tensor("wd2", [H, E], F32R, kind="ExternalInput").ap()
    bd2_d = nc.dram_tensor("bd2", [E], F32, kind="ExternalInput").ap()
    we_d = nc.dram_tensor("we", [E, H, H], F32R, kind="ExternalInput").ap()
    be_d = nc.dram_tensor("be", [E, H], F32R, kind="ExternalInput").ap()
    wi1_d = nc.dram_tensor("wi1", [H, H], BF16, kind="ExternalInput").ap()
    bi1_d = nc.dram_tensor("bi1", [H], F32, kind="ExternalInput").ap()
    wi2_d = nc.dram_tensor("wi2", [H, H], BF16, kind="ExternalInput").ap()
    bi2_d = nc.dram_tensor("bi2", [H], BF16, kind="ExternalInput").ap()
    out_d = nc.dram_tensor("out", [T, H], F32, kind="ExternalOutput").ap()

    wd1_re = wd1_d.rearrange("(k p) n -> p k n", p=P)
    wi1_re = wi1_d.rearrange("(k p) n -> p k n", p=P)
    wi2_re = wi2_d.rearrange("(k p) n -> p k n", p=P)

    with tile.TileContext(nc) as tc:
        with ExitStack() as ctx:
            const = ctx.enter_context(tc.tile_pool(name="const", bufs=1))
            bigp = ctx.enter_context(tc.tile_pool(name="bigp", bufs=1))
            wep = ctx.enter_context(tc.tile_pool(name="wep", bufs=2))
            hlp = ctx.enter_context(tc.tile_pool(name="hlp", bufs=2))
            a1p = ctx.enter_context(tc.tile_pool(name="a1p", bufs=2))
            tmpp = ctx.enter_context(tc.tile_pool(name="tmpp", bufs=2))
            opb = ctx.enter_context(tc.tile_pool(name="opb", bufs=3))
            osm = ctx.enter_context(tc.tile_pool(name="osm", bufs=3))
            pp = ctx.enter_context(tc.tile_pool(name="pp", bufs=3, space="PSUM"))

            # ---- input DMAs first so they win the queues at kernel start:
            # stage B starts on chunk 0 of Wd1 + the first half of h; with the
            # c-outer loop each Wd1 chunk serves 8 psums (~28us), so the DMA
            # deadlines for c1..c3 are trivially met.
            hls = []
            w1tiles = []
            for tt in range(TT):
                hl = hlp.tile([P, H], BF16, tag="hl", name=f"hl_{tt}")
                nc.sync.dma_start(hl, h_d[tt * P:(tt + 1) * P, :])
                hls.append(hl)
                if tt == TT // 2 - 1:
                    w1c = wep.tile([P, KT, 512], F32R, tag="we", name="wd1c_0")
                    nc.sync.dma_start(w1c, wd1_re[:, :, 0:512])
                    w1tiles.append(w1c)
            for c in range(1, C):
                w1c = wep.tile([P, KT, 512], F32R, tag="we", name=f"wd1c_{c}")
                nc.sync.dma_start(w1c, wd1_re[:, :, c * 512:(c + 1) * 512])
                w1tiles.append(w1c)

            # ---- engine-generated constants (no DMA) ----
            identF = const.tile([P, P], F32, name="identF")
            make_identity(nc, identF)
            ident = const.tile([P, P], BF16, name="ident")
            nc.scalar.copy(ident, identF)
            ones8f = const.tile([E, 1], F32, name="ones8f")
            nc.vector.memset(ones8f, 1.0)
            ones8 = const.tile([E, 1], F32R, name="ones8")
            nc.scalar.copy(ones8, ones8f)
            ones1x8f = const.tile([1, E], F32, name="ones1x8f")
            nc.vector.memset(ones1x8f, 1.0)
            ones1x8 = const.tile([1, E], F32R, name="ones1x8")
            nc.scalar.copy(ones1x8, ones1x8f)
            ones1xP = const.tile([1, P], BF16, name="ones1xP")
            nc.vector.memset(ones1xP, 1.0)
            # sel8[e', e*128+p] = (e' == e): K=8 selector used to broadcast
            # op_w rows across all 128 partitions via a tiny matmul.
            sel8f = const.tile([E, E, P], F32, name="sel8f")
            nc.gpsimd.memset(sel8f, 0.0)
            nc.gpsimd.affine_select(
                out=sel8f, in_=sel8f, compare_op=ALU.not_equal, fill=1.0,
                base=0, pattern=[[-1, E], [0, P]], channel_multiplier=1)
            sel8 = const.tile([E, E * P], F32R, name="sel8")
            nc.scalar.copy(sel8, sel8f.rearrange("e a p -> e (a p)"))

            # ---- small constant DMAs (after the h/wd1 loads) ----
            wd2_t = const.tile([P, KT, E], F32R, name="wd2_t")
            nc.sync.dma_start(wd2_t, wd2_d.rearrange("(k p) e -> p k e", p=P))
            bd1_t = const.tile([P, KT], F32, name="bd1_t")
            nc.sync.dma_start(bd1_t, bd1_d.rearrange("(k p) -> p k", p=P))
            bi1_t = const.tile([P, KT], F32, name="bi1_t")
            nc.sync.dma_start(bi1_t, bi1_d.rearrange("(k p) -> p k", p=P))
            bd2c = const.tile([E, 1], F32, name="bd2c")
            nc.sync.dma_start(bd2c, bd2_d.unsqueeze(1))
            mask_t = const.tile([P, TT], F32, name="mask_t")
            nc.sync.dma_start(mask_t, msk_d.rearrange("(t p) -> p t", p=P))
            if include_bi2:
                bi2_t = const.tile([1, H], BF16, name="bi2_t")
                nc.sync.dma_start(bi2_t, bi2_d.unsqueeze(0))
            if include_be:
                be_t = const.tile([E, H], F32R, name="be_t")
                nc.sync.dma_start(be_t, be_d)

            expT = const.tile([E, T], F32R, name="expT")
            # opwN overlays expT: the normalization multiply is in-place
            # (exp values are only read pre-normalization by the colsum).
            opwN = expT
            recip = const.tile([1, T], F32R, name="recip")

            # ---- batched PE transpose: 4 [128,128] tiles share one PSUM
            # bank, one batched eviction on an alternating engine ----
            ecnt = [0]

            def tbatch(srcs, out3):
                n = len(srcs)
                trp = pp.tile([P, 4, P], BF16, tag="tr", bufs=2, name="trb")
                for i, s in enumerate(srcs):
                    nc.tensor.matmul(trp[:, i, :], s, ident, is_transpose=True,
                                     start=(i == 0), stop=(i == n - 1))
                ecnt[0] += 1
                if ecnt[0] % 2 == 0:
                    nc.scalar.copy(out3, trp[:, :n, :])
                else:
                    nc.vector.tensor_copy(out3, trp[:, :n, :])

            # ---- stage A: transpose h to hT [H, T] (bf16) ----
            hT = bigp.tile([P, KT, T], F32R, tag="A", name="hT")

            def emit_tt_transposes(tt):
                for kg in range(KT // 4):
                    tbatch(
                        [hls[tt][:, (kg * 4 + j) * P:(kg * 4 + j + 1) * P]
                         for j in range(4)],
                        hT[:, kg * 4:kg * 4 + 4, tt * P:(tt + 1) * P])

            for tt in range(TT // 2):
                emit_tt_transposes(tt)
            deferred = [(TT // 2 + i) for i in range(TT - TT // 2)]

            # ---- stage B: a1 = gelu(Wd1.T @ hT + bd1) in [H_out, T], with
            # the logits GEMM accumulated in [E, T] PSUM across all m.
            # Loop order: c outer, tb middle, mi inner -- each Wd1 chunk is
            # consumed over ~28us so the chunk DMAs never stall the PE, and
            # the tb=1 column of chunk 0 gives the deferred transposes (which
            # interleave into the tb=0 column) time to finish.
            lgs = []

            def emit_b_m(tbp, idx, m, c, mi, lg):
                ps = pp.tile([P, 512], F32, tag="mm", bufs=4, name="ps_g1")
                for k in range(KT):
                    nc.tensor.matmul(ps, w1tiles[c][:, k, mi * P:(mi + 1) * P],
                                     hT[:, k, tbp * 512:(tbp + 1) * 512],
                                     start=(k == 0), stop=(k == KT - 1))
                a1 = a1p.tile([P, 512], F32R, tag="a1", name=f"a1_{tbp}_{m}")
                nc.scalar.activation(a1, ps, act, bias=bd1_t[:, m:m + 1])
                nc.tensor.matmul(lg, wd2_t[:, m, :], a1,
                                 start=(idx == 0), stop=(idx == MT - 1))

            def emit_softmax_tb(tb, lg):
                # exp(logits + bd2) on ACT: [E, 512]
                nc.scalar.activation(expT[:, tb * 512:(tb + 1) * 512], lg,
                                     AF.Exp, bias=bd2c, scale=1.0)

            def emit_colsum_tb(tb):
                cs = pp.tile([1, 512], F32, tag="lg", bufs=2, name=f"cs{tb}")
                nc.tensor.matmul(cs, ones8,
                                 expT[:, tb * 512:(tb + 1) * 512],
                                 start=True, stop=True)
                return cs

            def emit_recip_tb(tb, cs):
                # bf16 reciprocal: 2^-9 relative error on the softmax
                # normalizer is far inside the kernel's error budget.
                with nc.allow_low_precision(reason="softmax recip in bf16"):
                    nc.vector.reciprocal(recip[:, tb * 512:(tb + 1) * 512], cs)

            def emit_bcast_tb(tb):
                rb = pp.tile([E, 512], F32, tag="tr", bufs=2, name=f"rb{tb}")
                nc.tensor.matmul(rb, ones1x8,
                                 recip[:, tb * 512:(tb + 1) * 512],
                                 start=True, stop=True)
                nc.vector.tensor_tensor(
                    opwN[:, tb * 512:(tb + 1) * 512],
                    expT[:, tb * 512:(tb + 1) * 512], rb, op=ALU.mult)

            def emit_ob(e, tb):
                sp = pp.tile([P, 512], F32, tag="mm", bufs=4, name="sp")
                nc.tensor.matmul(sp, sel8[:, e * P:(e + 1) * P],
                                 opwN[:, tb * 512:(tb + 1) * 512],
                                 start=True, stop=True)
                ob = opb.tile([P, 512], F32, tag="ob", name=f"ob_{e}_{tb}")
                nc.scalar.copy(ob, sp)
                return ob

            lg0 = pp.tile([E, 512], F32, tag="lg", bufs=2, name="lg0")
            lg1 = pp.tile([E, 512], F32, tag="lg", bufs=2, name="lg1")
            lgs = [lg0, lg1]
            dq = [(tt, kg) for tt in deferred for kg in range(KT // 4)]
            sm0 = {}
            bidx = [0, 0]
            for c in range(C):
                for tb in range(TB):
                    for mi in range(4):
                        emit_b_m(tb, bidx[tb], c * 4 + mi, c, mi, lgs[tb])
                        bidx[tb] += 1
                        # deferred h transposes fill chunk 0's tb=0 column
                        for _ in range(4):
                            if dq:
                                tt, kg = dq.pop(0)
                                tbatch(
                                    [hls[tt][:, (kg * 4 + j) * P:
                                             (kg * 4 + j + 1) * P]
                                     for j in range(4)],
                                    hT[:, kg * 4:kg * 4 + 4,
                                       tt * P:(tt + 1) * P])
                        # T-half-0 softmax chain resolves inside chunk 3's
                        # tb=1 column so ob(0,0) is ready when experts start
                        if c == C - 1 and tb == 1:
                            if mi == 0:
                                emit_softmax_tb(0, lg0)
                            elif mi == 1:
                                sm0["cs"] = emit_colsum_tb(0)
                            elif mi == 2:
                                emit_recip_tb(0, sm0["cs"])
                                emit_bcast_tb(0)
                            elif mi == 3:
                                sm0["ob"] = emit_ob(0, 0)

            # ---- stage C: expert GEMMs + weighted combine into arena ----
            arena = bigp.tile([P, KT, TT, P], BF16, tag="B", name="arena")

            def emit_expert_psum(e, wec, mi, tb, m, ob):
                ps = pp.tile([P, 512], F32, tag="mm", bufs=4, name="eps")
                for k in range(KT):
                    nc.tensor.matmul(ps, wec[:, k, mi * P:(mi + 1) * P],
                                     hT[:, k, tb * 512:(tb + 1) * 512],
                                     start=(k == 0), stop=(k == KT - 1))
                wsl = arena[:, m, tb * 4:(tb + 1) * 4, :]
                ob3 = ob.rearrange("p (n c) -> p n c", c=P)
                ps3 = ps.rearrange("p (n c) -> p n c", c=P)
                if e == 0 and not include_be:
                    nc.vector.tensor_tensor(wsl, ps3, ob3, op=ALU.mult)
                else:
                    tmp = tmpp.tile([P, 512], F32, tag="t", name="tmp")
                    tmp3 = tmp.rearrange("p (n c) -> p n c", c=P)
                    nc.vector.tensor_tensor(tmp3, ps3, ob3, op=ALU.mult)
                    nc.vector.tensor_tensor(wsl, wsl, tmp3, op=ALU.add)

            def emit_be_init(tb):
                # arena[:, :, tb half] = sum_e op_w[t, e] * be[e, :]
                for m in range(MT):
                    bps = pp.tile([P, 512], F32, tag="mm", bufs=4, name="bps")
                    nc.tensor.matmul(bps, be_t[:, m * P:(m + 1) * P],
                                     opwN[:, tb * 512:(tb + 1) * 512],
                                     start=True, stop=True)
                    nc.scalar.copy(
                        arena[:, m, tb * 4:(tb + 1) * 4, :],
                        bps.rearrange("p (n c) -> p n c", c=P))

            obs = {0: sm0["ob"]}
            # T-half-1 softmax chain: exp1/cs1 go out right after stage B;
            # the rest (recip1 -> bcast -> ob(0,1)) is emitted after expert
            # 0's first psum stream so the DVE/ACT links resolve while that
            # psum fills.  The first combine needing ob(0,1) runs ~7us into
            # the expert phase.
            emit_softmax_tb(1, lg1)
            cs1 = emit_colsum_tb(1)

            def emit_sm1_rest():
                emit_recip_tb(1, cs1)
                emit_bcast_tb(1)
                obs[1] = emit_ob(0, 1)

            if include_be:
                emit_sm1_rest()
                for tb in range(TB):
                    emit_be_init(tb)
            for e in range(E):
                we_re = we_d[e].rearrange("(k p) n -> p k n", p=P)
                if e > 0:
                    for tb in range(TB):
                        obs[tb] = emit_ob(e, tb)
                for c in range(C):
                    wec = wep.tile([P, KT, 512], F32R, tag="we",
                                   name=f"we_{e}_{c}")
                    nc.sync.dma_start(wec,
                                      we_re[:, :, c * 512:(c + 1) * 512])
                    for mi in range(4):
                        for tb in range(TB):
                            emit_expert_psum(e, wec, mi, tb, c * 4 + mi,
                                             obs[tb])
                            if e == 0 and c == 0 and mi == 0 and tb == 0 \
                                    and 1 not in obs:
                                emit_sm1_rest()

            # ---- stage E: a2T = gelu(Wi1.T @ arena + bi1) [H, T] ----
            a2T = bigp.tile([P, KT, T], BF16, tag="A", name="a2T")
            for c in range(C):
                w3c = wep.tile([P, KT, 512], BF16, tag="we", name=f"wi1c_{c}")
                nc.sync.dma_start(w3c, wi1_re[:, :, c * 512:(c + 1) * 512])
                for mi in range(4):
                    m = c * 4 + mi
                    for tb in range(TB):
                        ps = pp.tile([P, 512], F32, tag="mm", bufs=4,
                                     name="ps_g3")
                        for k in range(KT):
                            nc.tensor.matmul(
                                ps, w3c[:, k, mi * P:(mi + 1) * P],
                                arena[:, k, tb * 4:(tb + 1) * 4, :],
                                start=(k == 0), stop=(k == KT - 1))
                        nc.scalar.activation(
                            a2T[:, m, tb * 512:(tb + 1) * 512], ps, act,
                            bias=bi1_t[:, m:m + 1])

            # ---- stage F (flipped): out[t, n] = a2.T @ Wi2 + bi2, PSUM in
            # [T, H] orientation, mask fused into the eviction, direct DMA ----
            for nb in range(C):
                w4c = wep.tile([P, KT, 512], BF16, tag="we", name=f"wi2c_{nb}")
                nc.sync.dma_start(w4c, wi2_re[:, :, nb * 512:(nb + 1) * 512])
                for tt in range(TT):
                    ps = pp.tile([P, 512], F32, tag="mm", bufs=4, name="ps_g4")
                    if include_bi2:
                        nc.tensor.matmul(ps, ones1xP,
                                         bi2_t[:, nb * 512:(nb + 1) * 512],
                                         start=True, stop=False)
                    for k in range(KT):
                        nc.tensor.matmul(
                            ps, a2T[:, k, tt * P:(tt + 1) * P],
                            w4c[:, k, :],
                            start=(k == 0 and not include_bi2),
                            stop=(k == KT - 1))
                    ot = osm.tile([P, 512], F32, tag="os", name="ot")
                    nc.scalar.activation(ot, ps, AF.Copy,
                                         scale=mask_t[:, tt:tt + 1])
                    nc.sync.dma_start(
                        out_d[tt * P:(tt + 1) * P, nb * 512:(nb + 1) * 512],
                        ot)

    nc.compile()
    return nc


_CACHED = {}


def _get_nc(T, H, E, include_be, include_bi2):
    key = (T, H, E, include_be, include_bi2)
    if key not in _CACHED:
        _CACHED[key] = build_nc(T, H, E, act=AF.Gelu, include_be=include_be,
                                include_bi2=include_bi2)
    return _CACHED[key]


def kernel(hidden_states, attention_mask, Wd1, bd1, Wd2, bd2, We, be, Wi1, bi1,
           Wi2, bi2, _trace=False):
    bf = lambda x: np.ascontiguousarray(
        np.asarray(x, dtype=np.float32).astype(ml_dtypes.bfloat16))
    f32 = lambda x: np.ascontiguousarray(np.asarray(x, dtype=np.float32))
    h = bf(hidden_states)
    mask = f32(attention_mask)
    Wd1b, bd1f, Wd2b, bd2f = f32(Wd1), f32(bd1), f32(Wd2), f32(bd2)
    Web, beb = f32(We), f32(be)
    Wi1b, bi1f, Wi2b, bi2b = bf(Wi1), f32(bi1), bf(Wi2), bf(bi2)

    Bv, Sv, Hv = h.shape
    Ev = Wd2b.shape[1]
    TOK = Bv * Sv
    T = TOK // N_CORES
    include_be = bool(np.any(np.asarray(be)))
    include_bi2 = bool(np.any(np.asarray(bi2)))

    nc = _get_nc(T, Hv, Ev, include_be, include_bi2)

    hf = h.reshape(TOK, Hv)
    mf = mask.reshape(TOK)
    weights = dict(wd1=Wd1b, bd1=bd1f, wd2=Wd2b, bd2=bd2f, we=Web, be=beb,
                   wi1=Wi1b, bi1=bi1f, wi2=Wi2b, bi2=bi2b)
    in_maps = []
    for c in range(N_CORES):
        m = dict(weights)
        m["h"] = np.ascontiguousarray(hf[c * T:(c + 1) * T])
        m["mask"] = np.ascontiguousarray(mf[c * T:(c + 1) * T])
        in_maps.append(m)

    # The first execution of a freshly-loaded NEFF occasionally trips a
    # transient NRT_EXEC_UNIT_UNRECOVERABLE on the axon worker; a retry after a
    # short pause has always succeeded, so tolerate a couple of those.
    last_exc = None
    for attempt in range(3):
        try:
            res = run_bass_kernel_spmd(nc, in_maps,
                                       core_ids=list(range(N_CORES)),
                                       trace=_trace)
            break
        except Exception as e:  # noqa: BLE001 - jax.errors.JaxRuntimeError
            last_exc = e
            if "UNAVAILABLE" not in str(e) and "unrecoverable" not in str(e):
                raise
            import time as _time
            _time.sleep(5 * (attempt + 1))
    else:
        raise last_exc
    out = np.concatenate([res.results[c]["out"] for c in range(N_CORES)], axis=0)
    out = out.reshape(Bv, Sv, Hv).astype(np.float32)
    if _trace:
        kernel._last_results = res
    return out


# revision 16
# speedup vs baseline: 1.2461x; 1.2461x over previous
"""Trainium2 Bass kernel for the EnhancedMathematicalReasoning MoE-routing module.

Computation (per token t, hidden dim H=2048, E=8 experts, dense routing):
    a1     = gelu(h @ Wd1 + bd1)
    logits = a1 @ Wd2 + bd2
    op_w   = softmax(logits)
    comb   = sum_e op_w[:, e] * (h @ We[e] + be[e])
    out    = (gelu(comb @ Wi1 + bi1) @ Wi2 + bi2) * mask

Sharding: data-parallel over the 8192 tokens -> 1024 tokens per NeuronCore,
weights replicated, no collectives.

Per-core strategy (P=128), v2 -- all GEMM operands in bf16:
  - Host casts h + all weights to bf16.  PE streaming rate is identical to
    f32r (1 cycle/row), but bf16 enables FWL fast weight loads (the f32r
    LDWEIGHTS was the exposed +14.5ns/MM tax in v1), halves all DMA traffic
    and makes PE transposes 1.0 cyc/row instead of 1.5.
  - h is PE-transposed once to hT [H, T] (bf16).  GEMM1/experts/GEMM3 run
    with the weight m-chunk stationary and a resident [H,*] activation as a
    512-wide moving operand, accumulating over K=16 chunks in PSUM.
  - Logits are accumulated in [E, T] orientation in a single PSUM bank per
    T-half across all 16 m-chunks (stationary = Wd2 m-slice [128, 8]), so
    softmax needs no transposes: exp on ACT (bias=bd2, no max subtraction --
    logits have sigma ~0.6), partition-sum + reciprocal-broadcast via tiny
    K=8/K=1 matmuls, one DVE multiply -> normalized op_w in [E, T].
  - Expert combine: op_w row e is broadcast to 128 partitions by a K=8
    selector matmul; DVE does comb += psum * ob into a bf16 arena [H, T].
  - GEMM1 loops c-outer / T-half-inner so each Wd1 column chunk covers
    ~28us of PE work (DMA deadlines trivially met) and the kernel starts as
    soon as the first 512 tokens are transposed; the remaining h transposes
    interleave into chunk 0's first column of psums.
  - GEMM4 is flipped: stationary = a2T token-slice [128k, 128t], moving =
    Wi2 column-chunk -> PSUM is directly [T, H]-oriented, the attention
    mask is fused into the eviction as a per-partition ACT scale, and the
    result DMAs straight out.  No output transposes, ~1.5us tail.
"""

import numpy as np
from contextlib import ExitStack

import ml_dtypes

import concourse.bass as bass
import concourse.tile as tile
from concourse import bacc, mybir
from concourse.bass_utils import run_bass_kernel_spmd
from concourse.masks import make_identity

F32 = mybir.dt.float32
BF16 = mybir.dt.bfloat16
AF = mybir.ActivationFunctionType
ALU = mybir.AluOpType
AX = mybir.AxisListType

P = 128
N_CORES = 8

B, S, H_FULL, E_FULL = 4, 2048, 2048, 8


def build_nc(T, H, E, act=AF.Gelu, include_be=False, include_bi2=False):
    """Build + compile the single-core program (same program runs SPMD on all
    cores). T: tokens per core. Requires T % 1024 == 0, H % 512 == 0."""
    assert T % 1024 == 0 and H % 512 == 0 and E <= P
    KT = H // P      # k-chunks of the contraction dim
    TT = T // P      # token 128-blocks
    TB = T // 512    # token 512-blocks
    MT = H // P      # output m-chunks
    C = H // 512     # 512-wide weight column chunks

    nc = bacc.Bacc("TRN2", target_bir_lowering=False, debug=False)

    h_d = nc.dram_tensor("h", [T, H], BF16, kind="ExternalInput").ap()
    msk_d = nc.dram_tensor("mask", [T], F32, kind="ExternalInput").ap()
    wd1_d = nc.dram_tensor("wd1", [H, H], BF16, kind="ExternalInput").ap()
    bd1_d = nc.dram_tensor("bd1", [H], F32, kind="ExternalInput").ap()
    wd2_d = nc.dram_tensor("wd2", [H, E], BF16, kind="ExternalInput").ap()
    bd2_d = nc.dram_tensor("bd2", [E], F32, kind="ExternalInput").ap()
    we_d = nc.dram_tensor("we", [E, H, H], BF16, kind="ExternalInput").ap()
    be_d = nc.dram_tensor("be", [E, H], BF16, kind="ExternalInput").ap()
    wi1_d = nc.dram_tensor("wi1", [H, H], BF16, kind="ExternalInput").ap()
    bi1_d = nc.dram_tensor("bi1", [H], F32, kind="ExternalInput").ap()
    wi2_d = nc.dram_tensor("wi2", [H, H], BF16, kind="ExternalInput").ap()
    bi2_d = nc.dram_tensor("bi2", [H], BF16, kind="ExternalInput").ap()
    out_d = nc.dram_tensor("out", [T, H], F32, kind="ExternalOutput").ap()

    wd1_re = wd1_d.rearrange("(k p) n -> p k n", p=P)
    wi1_re = wi1_d.rearrange("(k p) n -> p k n", p=P)
    wi2_re = wi2_d.rearrange("(k p) n -> p k n", p=P)

    with tile.TileContext(nc) as tc:
        with ExitStack() as ctx:
            const = ctx.enter_context(tc.tile_pool(name="const", bufs=1))
            bigp = ctx.enter_context(tc.tile_pool(name="bigp", bufs=1))
            wep = ctx.enter_context(tc.tile_pool(name="wep", bufs=4))
            hlp = ctx.enter_context(tc.tile_pool(name="hlp", bufs=4))
            a1p = ctx.enter_context(tc.tile_pool(name="a1p", bufs=3))
            tmpp = ctx.enter_context(tc.tile_pool(name="tmpp", bufs=3))
            opb = ctx.enter_context(tc.tile_pool(name="opb", bufs=4))
            osm = ctx.enter_context(tc.tile_pool(name="osm", bufs=4))
            pp = ctx.enter_context(tc.tile_pool(name="pp", bufs=3, space="PSUM"))

            # ---- input DMAs first so they win the queues at kernel start:
            # stage B starts on chunk 0 of Wd1 + the first half of h; with the
            # c-outer loop each Wd1 chunk serves 8 psums (~28us), so the DMA
            # deadlines for c1..c3 are trivially met.
            hls = []
            w1tiles = []
            for tt in range(TT):
                hl = hlp.tile([P, H], BF16, tag="hl", name=f"hl_{tt}")
                nc.sync.dma_start(hl, h_d[tt * P:(tt + 1) * P, :])
                hls.append(hl)
                if tt == TT // 2 - 1:
                    w1c = wep.tile([P, KT, 512], BF16, tag="we", name="wd1c_0")
                    nc.sync.dma_start(w1c, wd1_re[:, :, 0:512])
                    w1tiles.append(w1c)
            for c in range(1, C):
                w1c = wep.tile([P, KT, 512], BF16, tag="we", name=f"wd1c_{c}")
                nc.sync.dma_start(w1c, wd1_re[:, :, c * 512:(c + 1) * 512])
                w1tiles.append(w1c)

            # ---- engine-generated constants (no DMA) ----
            identF = const.tile([P, P], F32, name="identF")
            make_identity(nc, identF)
            ident = const.tile([P, P], BF16, name="ident")
            nc.scalar.copy(ident, identF)
            ones8 = const.tile([E, 1], BF16, name="ones8")
            nc.vector.memset(ones8, 1.0)
            ones1x8 = const.tile([1, E], BF16, name="ones1x8")
            nc.vector.memset(ones1x8, 1.0)
            ones1xP = const.tile([1, P], BF16, name="ones1xP")
            nc.vector.memset(ones1xP, 1.0)
            # sel8[e', e*128+p] = (e' == e): K=8 selector used to broadcast
            # op_w rows across all 128 partitions via a tiny matmul.
            sel8f = const.tile([E, E, P], F32, name="sel8f")
            nc.gpsimd.memset(sel8f, 0.0)
            nc.gpsimd.affine_select(
                out=sel8f, in_=sel8f, compare_op=ALU.not_equal, fill=1.0,
                base=0, pattern=[[-1, E], [0, P]], channel_multiplier=1)
            sel8 = const.tile([E, E * P], BF16, name="sel8")
            nc.scalar.copy(sel8, sel8f.rearrange("e a p -> e (a p)"))

            # ---- small constant DMAs (after the h/wd1 loads) ----
            wd2_t = const.tile([P, KT, E], BF16, name="wd2_t")
            nc.sync.dma_start(wd2_t, wd2_d.rearrange("(k p) e -> p k e", p=P))
            bd1_t = const.tile([P, KT], F32, name="bd1_t")
            nc.sync.dma_start(bd1_t, bd1_d.rearrange("(k p) -> p k", p=P))
            bi1_t = const.tile([P, KT], F32, name="bi1_t")
            nc.sync.dma_start(bi1_t, bi1_d.rearrange("(k p) -> p k", p=P))
            bd2c = const.tile([E, 1], F32, name="bd2c")
            nc.sync.dma_start(bd2c, bd2_d.unsqueeze(1))
            mask_t = const.tile([P, TT], F32, name="mask_t")
            nc.sync.dma_start(mask_t, msk_d.rearrange("(t p) -> p t", p=P))
            if include_bi2:
                bi2_t = const.tile([1, H], BF16, name="bi2_t")
                nc.sync.dma_start(bi2_t, bi2_d.unsqueeze(0))
            if include_be:
                be_t = const.tile([E, H], BF16, name="be_t")
                nc.sync.dma_start(be_t, be_d)

            expT = const.tile([E, T], BF16, name="expT")
            opwN = const.tile([E, T], BF16, name="opwN")
            recip = const.tile([1, T], BF16, name="recip")

            # ---- batched PE transpose: 4 [128,128] tiles share one PSUM
            # bank, one batched eviction on an alternating engine ----
            ecnt = [0]

            def tbatch(srcs, out3):
                n = len(srcs)
                trp = pp.tile([P, 4, P], BF16, tag="tr", bufs=2, name="trb")
                for i, s in enumerate(srcs):
                    nc.tensor.matmul(trp[:, i, :], s, ident, is_transpose=True,
                                     start=(i == 0), stop=(i == n - 1))
                ecnt[0] += 1
                if ecnt[0] % 2 == 0:
                    nc.scalar.copy(out3, trp[:, :n, :])
                else:
                    nc.vector.tensor_copy(out3, trp[:, :n, :])

            # ---- stage A: transpose h to hT [H, T] (bf16) ----
            hT = bigp.tile([P, KT, T], BF16, tag="A", name="hT")

            def emit_tt_transposes(tt):
                for kg in range(KT // 4):
                    tbatch(
                        [hls[tt][:, (kg * 4 + j) * P:(kg * 4 + j + 1) * P]
                         for j in range(4)],
                        hT[:, kg * 4:kg * 4 + 4, tt * P:(tt + 1) * P])

            for tt in range(TT // 2):
                emit_tt_transposes(tt)
            deferred = [(TT // 2 + i) for i in range(TT - TT // 2)]

            # ---- stage B: a1 = gelu(Wd1.T @ hT + bd1) in [H_out, T], with
            # the logits GEMM accumulated in [E, T] PSUM across all m.
            # Loop order: c outer, tb middle, mi inner -- each Wd1 chunk is
            # consumed over ~28us so the chunk DMAs never stall the PE, and
            # the tb=1 column of chunk 0 gives the deferred transposes (which
            # interleave into the tb=0 column) time to finish.
            lgs = []

            def emit_b_m(tbp, idx, m, c, mi, lg):
                ps = pp.tile([P, 512], F32, tag="mm", bufs=4, name="ps_g1")
                for k in range(KT):
                    nc.tensor.matmul(ps, w1tiles[c][:, k, mi * P:(mi + 1) * P],
                                     hT[:, k, tbp * 512:(tbp + 1) * 512],
                                     start=(k == 0), stop=(k == KT - 1))
                a1 = a1p.tile([P, 512], BF16, tag="a1", name=f"a1_{tbp}_{m}")
                nc.scalar.activation(a1, ps, act, bias=bd1_t[:, m:m + 1])
                nc.tensor.matmul(lg, wd2_t[:, m, :], a1,
                                 start=(idx == 0), stop=(idx == MT - 1))

            def emit_softmax_tb(tb, lg):
                # exp(logits + bd2) on ACT: [E, 512]
                nc.scalar.activation(expT[:, tb * 512:(tb + 1) * 512], lg,
                                     AF.Exp, bias=bd2c, scale=1.0)

            def emit_colsum_tb(tb):
                cs = pp.tile([1, 512], F32, tag="lg", bufs=2, name=f"cs{tb}")
                nc.tensor.matmul(cs, ones8,
                                 expT[:, tb * 512:(tb + 1) * 512],
                                 start=True, stop=True)
                return cs

            def emit_recip_tb(tb, cs):
                # bf16 reciprocal: 2^-9 relative error on the softmax
                # normalizer is far inside the kernel's error budget.
                with nc.allow_low_precision(reason="softmax recip in bf16"):
                    nc.vector.reciprocal(recip[:, tb * 512:(tb + 1) * 512], cs)

            def emit_bcast_tb(tb):
                rb = pp.tile([E, 512], F32, tag="tr", bufs=2, name=f"rb{tb}")
                nc.tensor.matmul(rb, ones1x8,
                                 recip[:, tb * 512:(tb + 1) * 512],
                                 start=True, stop=True)
                nc.vector.tensor_tensor(
                    opwN[:, tb * 512:(tb + 1) * 512],
                    expT[:, tb * 512:(tb + 1) * 512], rb, op=ALU.mult)

            def emit_ob(e, tb):
                sp = pp.tile([P, 512], F32, tag="mm", bufs=4, name="sp")
                nc.tensor.matmul(sp, sel8[:, e * P:(e + 1) * P],
                                 opwN[:, tb * 512:(tb + 1) * 512],
                                 start=True, stop=True)
                ob = opb.tile([P, 512], F32, tag="ob", name=f"ob_{e}_{tb}")
                nc.scalar.copy(ob, sp)
                return ob

            lg0 = pp.tile([E, 512], F32, tag="lg", bufs=2, name="lg0")
            lg1 = pp.tile([E, 512], F32, tag="lg", bufs=2, name="lg1")
            lgs = [lg0, lg1]
            dq = [(tt, kg) for tt in deferred for kg in range(KT // 4)]
            sm0 = {}
            bidx = [0, 0]
            for c in range(C):
                for tb in range(TB):
                    for mi in range(4):
                        emit_b_m(tb, bidx[tb], c * 4 + mi, c, mi, lgs[tb])
                        bidx[tb] += 1
                        # deferred h transposes fill chunk 0's tb=0 column
                        for _ in range(4):
                            if dq:
                                tt, kg = dq.pop(0)
                                tbatch(
                                    [hls[tt][:, (kg * 4 + j) * P:
                                             (kg * 4 + j + 1) * P]
                                     for j in range(4)],
                                    hT[:, kg * 4:kg * 4 + 4,
                                       tt * P:(tt + 1) * P])
                        # T-half-0 softmax chain resolves inside chunk 3's
                        # tb=1 column so ob(0,0) is ready when experts start
                        if c == C - 1 and tb == 1:
                            if mi == 0:
                                emit_softmax_tb(0, lg0)
                            elif mi == 1:
                                sm0["cs"] = emit_colsum_tb(0)
                            elif mi == 2:
                                emit_recip_tb(0, sm0["cs"])
                                emit_bcast_tb(0)
                            elif mi == 3:
                                sm0["ob"] = emit_ob(0, 0)

            # ---- stage C: expert GEMMs + weighted combine into arena ----
            arena = bigp.tile([P, KT, TT, P], BF16, tag="B", name="arena")

            def emit_expert_psum(e, wec, mi, tb, m, ob):
                ps = pp.tile([P, 512], F32, tag="mm", bufs=4, name="eps")
                for k in range(KT):
                    nc.tensor.matmul(ps, wec[:, k, mi * P:(mi + 1) * P],
                                     hT[:, k, tb * 512:(tb + 1) * 512],
                                     start=(k == 0), stop=(k == KT - 1))
                wsl = arena[:, m, tb * 4:(tb + 1) * 4, :]
                ob3 = ob.rearrange("p (n c) -> p n c", c=P)
                ps3 = ps.rearrange("p (n c) -> p n c", c=P)
                if e == 0 and not include_be:
                    nc.vector.tensor_tensor(wsl, ps3, ob3, op=ALU.mult)
                else:
                    tmp = tmpp.tile([P, 512], F32, tag="t", name="tmp")
                    tmp3 = tmp.rearrange("p (n c) -> p n c", c=P)
                    nc.vector.tensor_tensor(tmp3, ps3, ob3, op=ALU.mult)
                    nc.vector.tensor_tensor(wsl, wsl, tmp3, op=ALU.add)

            def emit_be_init(tb):
                # arena[:, :, tb half] = sum_e op_w[t, e] * be[e, :]
                for m in range(MT):
                    bps = pp.tile([P, 512], F32, tag="mm", bufs=4, name="bps")
                    nc.tensor.matmul(bps, be_t[:, m * P:(m + 1) * P],
                                     opwN[:, tb * 512:(tb + 1) * 512],
                                     start=True, stop=True)
                    nc.scalar.copy(
                        arena[:, m, tb * 4:(tb + 1) * 4, :],
                        bps.rearrange("p (n c) -> p n c", c=P))

            obs = {0: sm0["ob"]}
            # T-half-1 softmax chain: exp1/cs1 go out right after stage B;
            # the rest (recip1 -> bcast -> ob(0,1)) is emitted after expert
            # 0's first psum stream so the DVE/ACT links resolve while that
            # psum fills.  The first combine needing ob(0,1) runs ~7us into
            # the expert phase.
            emit_softmax_tb(1, lg1)
            cs1 = emit_colsum_tb(1)

            def emit_sm1_rest():
                emit_recip_tb(1, cs1)
                emit_bcast_tb(1)
                obs[1] = emit_ob(0, 1)

            if include_be:
                emit_sm1_rest()
                for tb in range(TB):
                    emit_be_init(tb)
            for e in range(E):
                we_re = we_d[e].rearrange("(k p) n -> p k n", p=P)
                if e > 0:
                    for tb in range(TB):
                        obs[tb] = emit_ob(e, tb)
                for c in range(C):
                    wec = wep.tile([P, KT, 512], BF16, tag="we",
                                   name=f"we_{e}_{c}")
                    nc.sync.dma_start(wec,
                                      we_re[:, :, c * 512:(c + 1) * 512])
                    for mi in range(4):
                        for tb in range(TB):
                            emit_expert_psum(e, wec, mi, tb, c * 4 + mi,
                                             obs[tb])
                            if e == 0 and c == 0 and mi == 0 and tb == 0 \
                                    and 1 not in obs:
                                emit_sm1_rest()

            # ---- stage E: a2T = gelu(Wi1.T @ arena + bi1) [H, T] ----
            a2T = bigp.tile([P, KT, T], BF16, tag="A", name="a2T")
            for c in range(C):
                w3c = wep.tile([P, KT, 512], BF16, tag="we", name=f"wi1c_{c}")
                nc.sync.dma_start(w3c, wi1_re[:, :, c * 512:(c + 1) * 512])
                for mi in range(4):
                    m = c * 4 + mi
                    for tb in range(TB):
                        ps = pp.tile([P, 512], F32, tag="mm", bufs=4,
                                     name="ps_g3")
                        for k in range(KT):
                            nc.tensor.matmul(
                                ps, w3c[:, k, mi * P:(mi + 1) * P],
                                arena[:, k, tb * 4:(tb + 1) * 4, :],
                                start=(k == 0), stop=(k == KT - 1))
                        nc.scalar.activation(
                            a2T[:, m, tb * 512:(tb + 1) * 512], ps, act,
                            bias=bi1_t[:, m:m + 1])

            # ---- stage F (flipped): out[t, n] = a2.T @ Wi2 + bi2, PSUM in
            # [T, H] orientation, mask fused into the eviction, direct DMA ----
            for nb in range(C):
                w4c = wep.tile([P, KT, 512], BF16, tag="we", name=f"wi2c_{nb}")
                nc.sync.dma_start(w4c, wi2_re[:, :, nb * 512:(nb + 1) * 512])
                for tt in range(TT):
                    ps = pp.tile([P, 512], F32, tag="mm", bufs=4, name="ps_g4")
                    if include_bi2:
                        nc.tensor.matmul(ps, ones1xP,
                                         bi2_t[:, nb * 512:(nb + 1) * 512],
                                         start=True, stop=False)
                    for k in range(KT):
                        nc.tensor.matmul(
                            ps, a2T[:, k, tt * P:(tt + 1) * P],
                            w4c[:, k, :],
                            start=(k == 0 and not include_bi2),
                            stop=(k == KT - 1))
                    ot = osm.tile([P, 512], F32, tag="os", name="ot")
                    nc.scalar.activation(ot, ps, AF.Copy,
                                         scale=mask_t[:, tt:tt + 1])
                    nc.sync.dma_start(
                        out_d[tt * P:(tt + 1) * P, nb * 512:(nb + 1) * 512],
                        ot)

    nc.compile()
    return nc


_CACHED = {}


def _get_nc(T, H, E, include_be, include_bi2):
    key = (T, H, E, include_be, include_bi2)
    if key not in _CACHED:
        _CACHED[key] = build_nc(T, H, E, act=AF.Gelu, include_be=include_be,
                                include_bi2=include_bi2)
    return _CACHED[key]


def kernel(hidden_states, attention_mask, Wd1, bd1, Wd2, bd2, We, be, Wi1, bi1,
           Wi2, bi2, _trace=False):
    bf = lambda x: np.ascontiguousarray(
        np.asarray(x, dtype=np.float32).astype(ml_dtypes.bfloat16))
    f32 = lambda x: np.ascontiguousarray(np.asarray(x, dtype=np.float32))
    h = bf(hidden_states)
    mask = f32(attention_mask)
    Wd1b, bd1f, Wd2b, bd2f = bf(Wd1), f32(bd1), bf(Wd2), f32(bd2)
    Web, beb = bf(We), bf(be)
    Wi1b, bi1f, Wi2b, bi2b = bf(Wi1), f32(bi1), bf(Wi2), bf(bi2)

    Bv, Sv, Hv = h.shape
    Ev = Wd2b.shape[1]
    TOK = Bv * Sv
    T = TOK // N_CORES
    include_be = bool(np.any(np.asarray(be)))
    include_bi2 = bool(np.any(np.asarray(bi2)))

    nc = _get_nc(T, Hv, Ev, include_be, include_bi2)

    hf = h.reshape(TOK, Hv)
    mf = mask.reshape(TOK)
    weights = dict(wd1=Wd1b, bd1=bd1f, wd2=Wd2b, bd2=bd2f, we=Web, be=beb,
                   wi1=Wi1b, bi1=bi1f, wi2=Wi2b, bi2=bi2b)
    in_maps = []
    for c in range(N_CORES):
        m = dict(weights)
        m["h"] = np.ascontiguousarray(hf[c * T:(c + 1) * T])
        m["mask"] = np.ascontiguousarray(mf[c * T:(c + 1) * T])
        in_maps.append(m)

    # The first execution of a freshly-loaded NEFF occasionally trips a
    # transient NRT_EXEC_UNIT_UNRECOVERABLE on the axon worker; a retry after a
    # short pause has always succeeded, so tolerate a couple of those.
    last_exc = None
    for attempt in range(3):
        try:
            res = run_bass_kernel_spmd(nc, in_maps,
                                       core_ids=list(range(N_CORES)),
                                       trace=_trace)
            break
        except Exception as e:  # noqa: BLE001 - jax.errors.JaxRuntimeError
            last_exc = e
            if "UNAVAILABLE" not in str(e) and "unrecoverable" not in str(e):
                raise
            import time as _time
            _time.sleep(5 * (attempt + 1))
    else:
        raise last_exc
    out = np.concatenate([res.results[c]["out"] for c in range(N_CORES)], axis=0)
    out = out.reshape(Bv, Sv, Hv).astype(np.float32)
    if _trace:
        kernel._last_results = res
    return out


# revision 18
# speedup vs baseline: 1.2471x; 1.0008x over previous
"""Trainium2 Bass kernel for the EnhancedMathematicalReasoning MoE-routing module.

Computation (per token t, hidden dim H=2048, E=8 experts, dense routing):
    a1     = gelu(h @ Wd1 + bd1)
    logits = a1 @ Wd2 + bd2
    op_w   = softmax(logits)
    comb   = sum_e op_w[:, e] * (h @ We[e] + be[e])
    out    = (gelu(comb @ Wi1 + bi1) @ Wi2 + bi2) * mask

Sharding: data-parallel over the 8192 tokens -> 1024 tokens per NeuronCore,
weights replicated, no collectives.

Per-core strategy (P=128), v2 -- all GEMM operands in bf16:
  - Host casts h + all weights to bf16.  PE streaming rate is identical to
    f32r (1 cycle/row), but bf16 enables FWL fast weight loads (the f32r
    LDWEIGHTS was the exposed +14.5ns/MM tax in v1), halves all DMA traffic
    and makes PE transposes 1.0 cyc/row instead of 1.5.
  - h is PE-transposed once to hT [H, T] (bf16).  GEMM1/experts/GEMM3 run
    with the weight m-chunk stationary and a resident [H,*] activation as a
    512-wide moving operand, accumulating over K=16 chunks in PSUM.
  - Logits are accumulated in [E, T] orientation in a single PSUM bank per
    T-half across all 16 m-chunks (stationary = Wd2 m-slice [128, 8]), so
    softmax needs no transposes: exp on ACT (bias=bd2, no max subtraction --
    logits have sigma ~0.6), partition-sum + reciprocal-broadcast via tiny
    K=8/K=1 matmuls, one DVE multiply -> normalized op_w in [E, T].
  - Expert combine: op_w row e is broadcast to 128 partitions by a K=8
    selector matmul; DVE does comb += psum * ob into a bf16 arena [H, T].
  - GEMM1 loops c-outer / T-half-inner so each Wd1 column chunk covers
    ~28us of PE work (DMA deadlines trivially met) and the kernel starts as
    soon as the first 512 tokens are transposed; the remaining h transposes
    interleave into chunk 0's first column of psums.
  - GEMM4 is flipped: stationary = a2T token-slice [128k, 128t], moving =
    Wi2 column-chunk -> PSUM is directly [T, H]-oriented, the attention
    mask is fused into the eviction as a per-partition ACT scale, and the
    result DMAs straight out.  No output transposes, ~1.5us tail.
"""

import numpy as np
from contextlib import ExitStack

import ml_dtypes

import concourse.bass as bass
import concourse.tile as tile
from concourse import bacc, mybir
from concourse.bass_utils import run_bass_kernel_spmd
from concourse.masks import make_identity

F32 = mybir.dt.float32
BF16 = mybir.dt.bfloat16
AF = mybir.ActivationFunctionType
ALU = mybir.AluOpType
AX = mybir.AxisListType

P = 128
N_CORES = 8

B, S, H_FULL, E_FULL = 4, 2048, 2048, 8


def build_nc(T, H, E, act=AF.Gelu, include_be=False, include_bi2=False):
    """Build + compile the single-core program (same program runs SPMD on all
    cores). T: tokens per core. Requires T % 1024 == 0, H % 512 == 0."""
    assert T % 1024 == 0 and H % 512 == 0 and E <= P
    KT = H // P      # k-chunks of the contraction dim
    TT = T // P      # token 128-blocks
    TB = T // 512    # token 512-blocks
    MT = H // P      # output m-chunks
    C = H // 512     # 512-wide weight column chunks

    nc = bacc.Bacc("TRN2", target_bir_lowering=False, debug=False)

    h_d = nc.dram_tensor("h", [T, H], BF16, kind="ExternalInput").ap()
    msk_d = nc.dram_tensor("mask", [T], F32, kind="ExternalInput").ap()
    wd1_d = nc.dram_tensor("wd1", [H, H], BF16, kind="ExternalInput").ap()
    bd1_d = nc.dram_tensor("bd1", [H], F32, kind="ExternalInput").ap()
    wd2_d = nc.dram_tensor("wd2", [H, E], BF16, kind="ExternalInput").ap()
    bd2_d = nc.dram_tensor("bd2", [E], F32, kind="ExternalInput").ap()
    we_d = nc.dram_tensor("we", [E, H, H], BF16, kind="ExternalInput").ap()
    be_d = nc.dram_tensor("be", [E, H], BF16, kind="ExternalInput").ap()
    wi1_d = nc.dram_tensor("wi1", [H, H], BF16, kind="ExternalInput").ap()
    bi1_d = nc.dram_tensor("bi1", [H], F32, kind="ExternalInput").ap()
    wi2_d = nc.dram_tensor("wi2", [H, H], BF16, kind="ExternalInput").ap()
    bi2_d = nc.dram_tensor("bi2", [H], BF16, kind="ExternalInput").ap()
    out_d = nc.dram_tensor("out", [T, H], F32, kind="ExternalOutput").ap()

    wd1_re = wd1_d.rearrange("(k p) n -> p k n", p=P)
    wi1_re = wi1_d.rearrange("(k p) n -> p k n", p=P)
    wi2_re = wi2_d.rearrange("(k p) n -> p k n", p=P)

    with tile.TileContext(nc) as tc:
        with ExitStack() as ctx:
            const = ctx.enter_context(tc.tile_pool(name="const", bufs=1))
            bigp = ctx.enter_context(tc.tile_pool(name="bigp", bufs=1))
            wep = ctx.enter_context(tc.tile_pool(name="wep", bufs=5))
            hlp = ctx.enter_context(tc.tile_pool(name="hlp", bufs=4))
            a1p = ctx.enter_context(tc.tile_pool(name="a1p", bufs=3))
            tmpp = ctx.enter_context(tc.tile_pool(name="tmpp", bufs=3))
            opb = ctx.enter_context(tc.tile_pool(name="opb", bufs=4))
            osm = ctx.enter_context(tc.tile_pool(name="osm", bufs=4))
            pp = ctx.enter_context(tc.tile_pool(name="pp", bufs=3, space="PSUM"))

            # ---- input DMAs first so they win the queues at kernel start:
            # stage B starts on chunk 0 of Wd1 + the first half of h; with the
            # c-outer loop each Wd1 chunk serves 8 psums (~28us), so the DMA
            # deadlines for c1..c3 are trivially met.
            hls = []
            w1tiles = []
            for tt in range(TT):
                hl = hlp.tile([P, H], BF16, tag="hl", name=f"hl_{tt}")
                nc.sync.dma_start(hl, h_d[tt * P:(tt + 1) * P, :])
                hls.append(hl)
                if tt == TT // 2 - 1:
                    # chunks 0 and 1 ahead of the second half of h: B consumes
                    # them at 29us/chunk while the tt4-7 transposes (deferred
                    # into chunk 0's stream) only need h by ~35us.
                    for c in range(2):
                        w1c = wep.tile([P, KT, 512], BF16, tag="we",
                                       name=f"wd1c_{c}")
                        nc.sync.dma_start(w1c,
                                          wd1_re[:, :, c * 512:(c + 1) * 512])
                        w1tiles.append(w1c)
            for c in range(2, C):
                w1c = wep.tile([P, KT, 512], BF16, tag="we", name=f"wd1c_{c}")
                nc.sync.dma_start(w1c, wd1_re[:, :, c * 512:(c + 1) * 512])
                w1tiles.append(w1c)

            # ---- engine-generated constants (no DMA) ----
            identF = const.tile([P, P], F32, name="identF")
            make_identity(nc, identF)
            ident = const.tile([P, P], BF16, name="ident")
            nc.scalar.copy(ident, identF)
            ones8 = const.tile([E, 1], BF16, name="ones8")
            nc.vector.memset(ones8, 1.0)
            ones1x8 = const.tile([1, E], BF16, name="ones1x8")
            nc.vector.memset(ones1x8, 1.0)
            ones1xP = const.tile([1, P], BF16, name="ones1xP")
            nc.vector.memset(ones1xP, 1.0)
            # sel8[e', e*128+p] = (e' == e): K=8 selector used to broadcast
            # op_w rows across all 128 partitions via a tiny matmul.
            sel8f = const.tile([E, E, P], F32, name="sel8f")
            nc.gpsimd.memset(sel8f, 0.0)
            nc.gpsimd.affine_select(
                out=sel8f, in_=sel8f, compare_op=ALU.not_equal, fill=1.0,
                base=0, pattern=[[-1, E], [0, P]], channel_multiplier=1)
            sel8 = const.tile([E, E * P], BF16, name="sel8")
            nc.scalar.copy(sel8, sel8f.rearrange("e a p -> e (a p)"))

            # ---- small constant DMAs (after the h/wd1 loads) ----
            wd2_t = const.tile([P, KT, E], BF16, name="wd2_t")
            nc.sync.dma_start(wd2_t, wd2_d.rearrange("(k p) e -> p k e", p=P))
            bd1_t = const.tile([P, KT], F32, name="bd1_t")
            nc.sync.dma_start(bd1_t, bd1_d.rearrange("(k p) -> p k", p=P))
            bi1_t = const.tile([P, KT], F32, name="bi1_t")
            nc.sync.dma_start(bi1_t, bi1_d.rearrange("(k p) -> p k", p=P))
            bd2c = const.tile([E, 1], F32, name="bd2c")
            nc.sync.dma_start(bd2c, bd2_d.unsqueeze(1))
            mask_t = const.tile([P, TT], F32, name="mask_t")
            nc.sync.dma_start(mask_t, msk_d.rearrange("(t p) -> p t", p=P))
            if include_bi2:
                bi2_t = const.tile([1, H], BF16, name="bi2_t")
                nc.sync.dma_start(bi2_t, bi2_d.unsqueeze(0))
            if include_be:
                be_t = const.tile([E, H], BF16, name="be_t")
                nc.sync.dma_start(be_t, be_d)

            expT = const.tile([E, T], BF16, name="expT")
            opwN = const.tile([E, T], BF16, name="opwN")
            recip = const.tile([1, T], BF16, name="recip")

            # ---- batched PE transpose: 4 [128,128] tiles share one PSUM
            # bank, one batched eviction on an alternating engine ----
            ecnt = [0]

            def tbatch(srcs, out3):
                n = len(srcs)
                trp = pp.tile([P, 4, P], BF16, tag="tr", bufs=2, name="trb")
                for i, s in enumerate(srcs):
                    nc.tensor.matmul(trp[:, i, :], s, ident, is_transpose=True,
                                     start=(i == 0), stop=(i == n - 1))
                ecnt[0] += 1
                if ecnt[0] % 2 == 0:
                    nc.scalar.copy(out3, trp[:, :n, :])
                else:
                    nc.vector.tensor_copy(out3, trp[:, :n, :])

            # ---- stage A: transpose h to hT [H, T] (bf16) ----
            hT = bigp.tile([P, KT, T], BF16, tag="A", name="hT")

            def emit_tt_transposes(tt):
                for kg in range(KT // 4):
                    tbatch(
                        [hls[tt][:, (kg * 4 + j) * P:(kg * 4 + j + 1) * P]
                         for j in range(4)],
                        hT[:, kg * 4:kg * 4 + 4, tt * P:(tt + 1) * P])

            for tt in range(TT // 2):
                emit_tt_transposes(tt)
            deferred = [(TT // 2 + i) for i in range(TT - TT // 2)]

            # ---- stage B: a1 = gelu(Wd1.T @ hT + bd1) in [H_out, T], with
            # the logits GEMM accumulated in [E, T] PSUM across all m.
            # Loop order: c outer, tb middle, mi inner -- each Wd1 chunk is
            # consumed over ~28us so the chunk DMAs never stall the PE, and
            # the tb=1 column of chunk 0 gives the deferred transposes (which
            # interleave into the tb=0 column) time to finish.
            lgs = []

            def emit_b_m(tbp, idx, m, c, mi, lg):
                ps = pp.tile([P, 512], F32, tag="mm", bufs=4, name="ps_g1")
                for k in range(KT):
                    nc.tensor.matmul(ps, w1tiles[c][:, k, mi * P:(mi + 1) * P],
                                     hT[:, k, tbp * 512:(tbp + 1) * 512],
                                     start=(k == 0), stop=(k == KT - 1))
                a1 = a1p.tile([P, 512], BF16, tag="a1", name=f"a1_{tbp}_{m}")
                nc.scalar.activation(a1, ps, act, bias=bd1_t[:, m:m + 1])
                nc.tensor.matmul(lg, wd2_t[:, m, :], a1,
                                 start=(idx == 0), stop=(idx == MT - 1))

            def emit_softmax_tb(tb, lg):
                # exp(logits + bd2) on ACT: [E, 512]
                nc.scalar.activation(expT[:, tb * 512:(tb + 1) * 512], lg,
                                     AF.Exp, bias=bd2c, scale=1.0)

            def emit_colsum_tb(tb):
                cs = pp.tile([1, 512], F32, tag="lg", bufs=2, name=f"cs{tb}")
                nc.tensor.matmul(cs, ones8,
                                 expT[:, tb * 512:(tb + 1) * 512],
                                 start=True, stop=True)
                return cs

            def emit_recip_tb(tb, cs):
                # bf16 reciprocal: 2^-9 relative error on the softmax
                # normalizer is far inside the kernel's error budget.
                with nc.allow_low_precision(reason="softmax recip in bf16"):
                    nc.vector.reciprocal(recip[:, tb * 512:(tb + 1) * 512], cs)

            def emit_bcast_tb(tb):
                rb = pp.tile([E, 512], F32, tag="tr", bufs=2, name=f"rb{tb}")
                nc.tensor.matmul(rb, ones1x8,
                                 recip[:, tb * 512:(tb + 1) * 512],
                                 start=True, stop=True)
                nc.vector.tensor_tensor(
                    opwN[:, tb * 512:(tb + 1) * 512],
                    expT[:, tb * 512:(tb + 1) * 512], rb, op=ALU.mult)

            def emit_ob(e, tb):
                sp = pp.tile([P, 512], F32, tag="mm", bufs=4, name="sp")
                nc.tensor.matmul(sp, sel8[:, e * P:(e + 1) * P],
                                 opwN[:, tb * 512:(tb + 1) * 512],
                                 start=True, stop=True)
                ob = opb.tile([P, 512], F32, tag="ob", name=f"ob_{e}_{tb}")
                nc.scalar.copy(ob, sp)
                return ob

            lg0 = pp.tile([E, 512], F32, tag="lg", bufs=2, name="lg0")
            lg1 = pp.tile([E, 512], F32, tag="lg", bufs=2, name="lg1")
            lgs = [lg0, lg1]
            dq = [(tt, kg) for tt in deferred for kg in range(KT // 4)]
            sm0 = {}
            bidx = [0, 0]
            for c in range(C):
                for tb in range(TB):
                    for mi in range(4):
                        emit_b_m(tb, bidx[tb], c * 4 + mi, c, mi, lgs[tb])
                        bidx[tb] += 1
                        # deferred h transposes fill chunk 0's tb=0 column
                        for _ in range(4):
                            if dq:
                                tt, kg = dq.pop(0)
                                tbatch(
                                    [hls[tt][:, (kg * 4 + j) * P:
                                             (kg * 4 + j + 1) * P]
                                     for j in range(4)],
                                    hT[:, kg * 4:kg * 4 + 4,
                                       tt * P:(tt + 1) * P])
                        # T-half-0 softmax chain resolves inside chunk 3's
                        # tb=1 column so ob(0,0) is ready when experts start
                        if c == C - 1 and tb == 1:
                            if mi == 0:
                                emit_softmax_tb(0, lg0)
                            elif mi == 1:
                                sm0["cs"] = emit_colsum_tb(0)
                            elif mi == 2:
                                emit_recip_tb(0, sm0["cs"])
                                emit_bcast_tb(0)
                            elif mi == 3:
                                sm0["ob"] = emit_ob(0, 0)

            # ---- stage C: expert GEMMs + weighted combine into arena ----
            arena = bigp.tile([P, KT, TT, P], BF16, tag="B", name="arena")

            def emit_expert_psum(e, wec, mi, tb, m, ob):
                ps = pp.tile([P, 512], F32, tag="mm", bufs=4, name="eps")
                for k in range(KT):
                    nc.tensor.matmul(ps, wec[:, k, mi * P:(mi + 1) * P],
                                     hT[:, k, tb * 512:(tb + 1) * 512],
                                     start=(k == 0), stop=(k == KT - 1))
                wsl = arena[:, m, tb * 4:(tb + 1) * 4, :]
                ob3 = ob.rearrange("p (n c) -> p n c", c=P)
                ps3 = ps.rearrange("p (n c) -> p n c", c=P)
                if e == 0 and not include_be:
                    nc.vector.tensor_tensor(wsl, ps3, ob3, op=ALU.mult)
                else:
                    tmp = tmpp.tile([P, 512], F32, tag="t", name="tmp")
                    tmp3 = tmp.rearrange("p (n c) -> p n c", c=P)
                    nc.vector.tensor_tensor(tmp3, ps3, ob3, op=ALU.mult)
                    nc.vector.tensor_tensor(wsl, wsl, tmp3, op=ALU.add)

            def emit_be_init(tb):
                # arena[:, :, tb half] = sum_e op_w[t, e] * be[e, :]
                for m in range(MT):
                    bps = pp.tile([P, 512], F32, tag="mm", bufs=4, name="bps")
                    nc.tensor.matmul(bps, be_t[:, m * P:(m + 1) * P],
                                     opwN[:, tb * 512:(tb + 1) * 512],
                                     start=True, stop=True)
                    nc.scalar.copy(
                        arena[:, m, tb * 4:(tb + 1) * 4, :],
                        bps.rearrange("p (n c) -> p n c", c=P))

            obs = {0: sm0["ob"]}
            # T-half-1 softmax chain: exp1/cs1 go out right after stage B;
            # the rest (recip1 -> bcast -> ob(0,1)) is emitted after expert
            # 0's first psum stream so the DVE/ACT links resolve while that
            # psum fills.  The first combine needing ob(0,1) runs ~7us into
            # the expert phase.
            emit_softmax_tb(1, lg1)
            cs1 = emit_colsum_tb(1)

            def emit_sm1_rest():
                emit_recip_tb(1, cs1)
                emit_bcast_tb(1)
                obs[1] = emit_ob(0, 1)

            if include_be:
                emit_sm1_rest()
                for tb in range(TB):
                    emit_be_init(tb)
            for e in range(E):
                we_re = we_d[e].rearrange("(k p) n -> p k n", p=P)
                if e > 0:
                    for tb in range(TB):
                        obs[tb] = emit_ob(e, tb)
                for c in range(C):
                    wec = wep.tile([P, KT, 512], BF16, tag="we",
                                   name=f"we_{e}_{c}")
                    nc.sync.dma_start(wec,
                                      we_re[:, :, c * 512:(c + 1) * 512])
                    for mi in range(4):
                        for tb in range(TB):
                            emit_expert_psum(e, wec, mi, tb, c * 4 + mi,
                                             obs[tb])
                            if e == 0 and c == 0 and mi == 0 and tb == 0 \
                                    and 1 not in obs:
                                emit_sm1_rest()

            # ---- stage E: a2T = gelu(Wi1.T @ arena + bi1) [H, T] ----
            a2T = bigp.tile([P, KT, T], BF16, tag="A", name="a2T")
            for c in range(C):
                w3c = wep.tile([P, KT, 512], BF16, tag="we", name=f"wi1c_{c}")
                nc.sync.dma_start(w3c, wi1_re[:, :, c * 512:(c + 1) * 512])
                for mi in range(4):
                    m = c * 4 + mi
                    for tb in range(TB):
                        ps = pp.tile([P, 512], F32, tag="mm", bufs=4,
                                     name="ps_g3")
                        for k in range(KT):
                            nc.tensor.matmul(
                                ps, w3c[:, k, mi * P:(mi + 1) * P],
                                arena[:, k, tb * 4:(tb + 1) * 4, :],
                                start=(k == 0), stop=(k == KT - 1))
                        nc.scalar.activation(
                            a2T[:, m, tb * 512:(tb + 1) * 512], ps, act,
                            bias=bi1_t[:, m:m + 1])

            # ---- stage F (flipped): out[t, n] = a2.T @ Wi2 + bi2, PSUM in
            # [T, H] orientation, mask fused into the eviction, direct DMA ----
            for nb in range(C):
                w4c = wep.tile([P, KT, 512], BF16, tag="we", name=f"wi2c_{nb}")
                nc.sync.dma_start(w4c, wi2_re[:, :, nb * 512:(nb + 1) * 512])
                for tt in range(TT):
                    ps = pp.tile([P, 512], F32, tag="mm", bufs=4, name="ps_g4")
                    if include_bi2:
                        nc.tensor.matmul(ps, ones1xP,
                                         bi2_t[:, nb * 512:(nb + 1) * 512],
                                         start=True, stop=False)
                    for k in range(KT):
                        nc.tensor.matmul(
                            ps, a2T[:, k, tt * P:(tt + 1) * P],
                            w4c[:, k, :],
                            start=(k == 0 and not include_bi2),
                            stop=(k == KT - 1))
                    ot = osm.tile([P, 512], F32, tag="os", name="ot")
                    if (nb * TT + tt) % 2 == 0:
                        nc.scalar.activation(ot, ps, AF.Copy,
                                             scale=mask_t[:, tt:tt + 1])
                    else:
                        nc.vector.tensor_scalar_mul(ot, ps,
                                                    mask_t[:, tt:tt + 1])
                    nc.sync.dma_start(
                        out_d[tt * P:(tt + 1) * P, nb * 512:(nb + 1) * 512],
                        ot)

    nc.compile()
    return nc


_CACHED = {}


def _get_nc(T, H, E, include_be, include_bi2):
    key = (T, H, E, include_be, include_bi2)
    if key not in _CACHED:
        _CACHED[key] = build_nc(T, H, E, act=AF.Gelu, include_be=include_be,
                                include_bi2=include_bi2)
    return _CACHED[key]


def kernel(hidden_states, attention_mask, Wd1, bd1, Wd2, bd2, We, be, Wi1, bi1,
           Wi2, bi2, _trace=False):
    bf = lambda x: np.ascontiguousarray(
        np.asarray(x, dtype=np.float32).astype(ml_dtypes.bfloat16))
    f32 = lambda x: np.ascontiguousarray(np.asarray(x, dtype=np.float32))
    h = bf(hidden_states)
    mask = f32(attention_mask)
    Wd1b, bd1f, Wd2b, bd2f = bf(Wd1), f32(bd1), bf(Wd2), f32(bd2)
    Web, beb = bf(We), bf(be)
    Wi1b, bi1f, Wi2b, bi2b = bf(Wi1), f32(bi1), bf(Wi2), bf(bi2)

    Bv, Sv, Hv = h.shape
    Ev = Wd2b.shape[1]
    TOK = Bv * Sv
    T = TOK // N_CORES
    include_be = bool(np.any(np.asarray(be)))
    include_bi2 = bool(np.any(np.asarray(bi2)))

    nc = _get_nc(T, Hv, Ev, include_be, include_bi2)

    hf = h.reshape(TOK, Hv)
    mf = mask.reshape(TOK)
    weights = dict(wd1=Wd1b, bd1=bd1f, wd2=Wd2b, bd2=bd2f, we=Web, be=beb,
                   wi1=Wi1b, bi1=bi1f, wi2=Wi2b, bi2=bi2b)
    in_maps = []
    for c in range(N_CORES):
        m = dict(weights)
        m["h"] = np.ascontiguousarray(hf[c * T:(c + 1) * T])
        m["mask"] = np.ascontiguousarray(mf[c * T:(c + 1) * T])
        in_maps.append(m)

    # The first execution of a freshly-loaded NEFF occasionally trips a
    # transient NRT_EXEC_UNIT_UNRECOVERABLE on the axon worker; a retry after a
    # short pause has always succeeded, so tolerate a couple of those.
    last_exc = None
    for attempt in range(3):
        try:
            res = run_bass_kernel_spmd(nc, in_maps,
                                       core_ids=list(range(N_CORES)),
                                       trace=_trace)
            break
        except Exception as e:  # noqa: BLE001 - jax.errors.JaxRuntimeError
            last_exc = e
            if "UNAVAILABLE" not in str(e) and "unrecoverable" not in str(e):
                raise
            import time as _time
            _time.sleep(5 * (attempt + 1))
    else:
        raise last_exc
    out = np.concatenate([res.results[c]["out"] for c in range(N_CORES)], axis=0)
    out = out.reshape(Bv, Sv, Hv).astype(np.float32)
    if _trace:
        kernel._last_results = res
    return out
